# revision 7
# baseline (speedup 1.0000x reference)
"""Trainium2 Bass kernel for a 3-layer GRU decoder (DecoderRNN).

Math (per timestep, identical input x0 each step):
    x0 = encoder_hidden @ w_proj.T + b_proj
    3 stacked GRU layers (PyTorch gate order r,z,n), then logits = h2 @ w_out.T + b_out

Device mapping (per core, batch shard BS=4096):
  - Layout: features on SBUF partitions, batch on the free dim.
  - All weights pre-transposed/packed on host; biases ride in an extra
    contraction row against a constant 1.0 row held in each state tile.
  - Gates accumulate in PSUM ([wih|whh] matmul pairs); sigmoid(r,z) is a
    single merged ACT op over a 2-bank PSUM tile; tanh/TT ops run on
    ACT/DVE over full-batch SBUF tensors.
  - w_out is packed into the layer-2 whh_n stationary, so logits(t-1) fall
    out of step t's PE pass for free; step 20's logits use one extra stream.
  - Output is written [T, VOCAB, BS] per core (2KB contiguous rows) and
    transposed back to [B, T, VOCAB] on the host.
"""

import numpy as np
import ml_dtypes

import concourse.bass as bass
import concourse.mybir as mybir
from concourse import bacc
from concourse.tile import TileContext
from concourse.bass_utils import run_bass_kernel_spmd

N_CORES = 8
B = 32768
BS = B // N_CORES  # 4096
LATENT = 128
H = 100
VOCAB = 20
T = 21
C = 512  # batch chunk = one PSUM bank of fp32

BF16 = mybir.dt.bfloat16
F32 = mybir.dt.float32
AF = mybir.ActivationFunctionType
OP = mybir.AluOpType


def build_nc(bs=BS, n_steps=T):
    nchunk = bs // C
    nc = bacc.Bacc("TRN2", target_bir_lowering=False)

    ehT = nc.declare_dram_parameter("ehT", [LATENT, bs], BF16, isOutput=False)
    wproj = nc.declare_dram_parameter("wproj", [LATENT, H], BF16, isOutput=False)
    wih = [
        nc.declare_dram_parameter(f"wih{l}", [H + 1, 3 * H], BF16, isOutput=False)
        for l in range(3)
    ]
    whh = [
        nc.declare_dram_parameter(f"whh{l}", [H + 1, 3 * H], BF16, isOutput=False)
        for l in range(3)
    ]
    # layer-2 n-gate stationary with w_out packed in cols 100:120
    wnout = nc.declare_dram_parameter("wnout", [H + 1, H + VOCAB], BF16, isOutput=False)
    out = nc.declare_dram_parameter("out", [n_steps, VOCAB + 4, bs], F32, isOutput=True)

    with TileContext(nc) as tc:
        with (
            tc.tile_pool(name="const", bufs=1) as cpool,
            tc.tile_pool(name="state", bufs=1) as spool,
            tc.tile_pool(name="rz", bufs=2) as rzpool,
            tc.tile_pool(name="work", bufs=2) as wpool,
            tc.tile_pool(name="psum", bufs=2, space="PSUM") as ppool,
        ):
            # ---- load weights ----
            eh_sb = cpool.tile([LATENT, bs], BF16, tag="eh")
            nc.sync.dma_start(eh_sb[:, :], ehT[:, :])
            wproj_sb = cpool.tile([LATENT, H], BF16, tag="wproj")
            nc.sync.dma_start(wproj_sb[:, :], wproj[:, :])
            wih_sb = []
            whh_sb = []
            for l in range(3):
                wi = cpool.tile([H + 1, 3 * H], BF16, tag=f"wih{l}")
                nc.sync.dma_start(wi[:, :], wih[l][:, :])
                wih_sb.append(wi)
                wh = cpool.tile([H + 1, 3 * H], BF16, tag=f"whh{l}")
                nc.sync.dma_start(wh[:, :], whh[l][:, :])
                whh_sb.append(wh)
            wnout_sb = cpool.tile([H + 1, H + VOCAB], BF16, tag="wnout")
            nc.sync.dma_start(wnout_sb[:, :], wnout[:, :])

            # ---- state tiles ----
            # ones-row writes start at partition 96 (HW requires 32-aligned
            # partition bases), then zero back rows 96:100
            h_sb = []
            for l in range(3):
                h = spool.tile([H + 1, bs], BF16, tag=f"h{l}")
                nc.gpsimd.memset(h[:, :], 0.0)
                nc.gpsimd.memset(h[96 : H + 1, :], 1.0)
                nc.gpsimd.memset(h[96:H, :], 0.0)
                h_sb.append(h)
            x0 = spool.tile([H + 1, bs], BF16, tag="x0")
            nc.gpsimd.memset(x0[96 : H + 1, :], 1.0)
            nc.gpsimd.memset(x0[96:H, :], 0.0)
            gxn0 = spool.tile([H, bs], BF16, tag="gxn0")

            # ---- prologue: x0 = wproj.T @ ehT ; gxn0 = wih0_n.T @ x0 ----
            for c in range(nchunk):
                sl = slice(c * C, (c + 1) * C)
                ps = ppool.tile([H, 2 * C], F32, tag="grz")
                nc.tensor.matmul(
                    ps[:, 0:C], wproj_sb[:, :], eh_sb[:, sl], start=True, stop=True
                )
                nc.scalar.copy(x0[0:H, sl], ps[:, 0:C])
            for c in range(nchunk):
                sl = slice(c * C, (c + 1) * C)
                ps = ppool.tile([H, 2 * C], F32, tag="grz")
                nc.tensor.matmul(
                    ps[:, 0:C],
                    wih_sb[0][:, 2 * H : 3 * H],
                    x0[:, sl],
                    start=True,
                    stop=True,
                )
                nc.scalar.copy(gxn0[:, sl], ps[:, 0:C])

            # ---- time loop ----
            for t in range(n_steps):
                for l in range(3):
                    h = h_sb[l]
                    hprev = x0 if l == 0 else h_sb[l - 1]
                    wi = wih_sb[l]
                    wh = whh_sb[l]

                    rz = rzpool.tile([H, 2 * bs], BF16, tag="rz")
                    t1 = wpool.tile([H, bs], BF16, tag="t1")
                    t2 = wpool.tile([H, bs], BF16, tag="t2")
                    nbuf = wpool.tile([H, bs], BF16, tag="n")
                    dbuf = wpool.tile([H, bs], BF16, tag="d")
                    ebuf = wpool.tile([H, bs], BF16, tag="e")
                    if l == 2 and t > 0:
                        lgbuf = wpool.tile([VOCAB + 4, bs], F32, tag="lg")

                    for c in range(nchunk):
                        sl = slice(c * C, (c + 1) * C)
                        grz = ppool.tile([H, 2 * C], F32, tag="grz")
                        # r gate (whh part then wih part, accumulated)
                        nc.tensor.matmul(
                            grz[:, 0:C], wh[:, 0:H], h[:, sl], start=True, stop=False
                        )
                        nc.tensor.matmul(
                            grz[:, 0:C], wi[:, 0:H], hprev[:, sl], start=False, stop=True
                        )
                        # z gate
                        nc.tensor.matmul(
                            grz[:, C : 2 * C],
                            wh[:, H : 2 * H],
                            h[:, sl],
                            start=True,
                            stop=False,
                        )
                        nc.tensor.matmul(
                            grz[:, C : 2 * C],
                            wi[:, H : 2 * H],
                            hprev[:, sl],
                            start=False,
                            stop=True,
                        )
                        # n gate hidden part (layer 2 also computes logits(t-1))
                        if l == 2:
                            ghn = ppool.tile([H + VOCAB, C], F32, tag="ghn")
                            nc.tensor.matmul(
                                ghn[:, :], wnout_sb[:, :], h[:, sl], start=True, stop=True
                            )
                            if t > 0:
                                nc.scalar.copy(lgbuf[:, sl], ghn[96 : H + VOCAB, :])
                        else:
                            ghn = ppool.tile([H + VOCAB, C], F32, tag="ghn")
                            nc.tensor.matmul(
                                ghn[0:H, :],
                                wh[:, 2 * H : 3 * H],
                                h[:, sl],
                                start=True,
                                stop=True,
                            )
                        # sigmoid over merged (r|z)
                        nc.scalar.activation(
                            rz[:, c * 2 * C : (c + 1) * 2 * C], grz[:, :], AF.Sigmoid
                        )
                        # t1 = r * ghn
                        nc.vector.tensor_mul(
                            t1[:, sl], rz[:, c * 2 * C : c * 2 * C + C], ghn[0:H, :]
                        )
                        if l >= 1:
                            gxn = ppool.tile([H, C], F32, tag="gxn")
                            nc.tensor.matmul(
                                gxn[:, :],
                                wi[:, 2 * H : 3 * H],
                                hprev[:, sl],
                                start=True,
                                stop=True,
                            )
                            nc.vector.tensor_add(t2[:, sl], t1[:, sl], gxn[:, :])
                    if l == 0:
                        nc.vector.tensor_add(t2[:, :], t1[:, :], gxn0[:, :])
                    nc.scalar.activation(nbuf[:, :], t2[:, :], AF.Tanh)
                    # h' = n + z*(h - n)
                    nc.vector.tensor_sub(dbuf[:, :], h[0:H, :], nbuf[:, :])
                    z3 = rz[:, :].rearrange("p (a b) -> p a b", b=2 * C)[:, :, C : 2 * C]
                    d3 = dbuf[:, :].rearrange("p (a b) -> p a b", b=C)
                    e3 = ebuf[:, :].rearrange("p (a b) -> p a b", b=C)
                    nc.vector.tensor_mul(e3, z3, d3)
                    nc.vector.tensor_add(h[0:H, :], nbuf[:, :], ebuf[:, :])
                    if l == 2 and t > 0:
                        nc.sync.dma_start(out[t - 1, :, :], lgbuf[:, :])

            # ---- epilogue: logits for the last step ----
            lgbuf = wpool.tile([VOCAB + 4, bs], F32, tag="lg")
            for c in range(nchunk):
                sl = slice(c * C, (c + 1) * C)
                lg = ppool.tile([H + VOCAB, C], F32, tag="ghn")
                nc.tensor.matmul(
                    lg[0 : VOCAB + 4, :],
                    wnout_sb[:, 96 : H + VOCAB],
                    h_sb[2][:, sl],
                    start=True,
                    stop=True,
                )
                nc.scalar.copy(lgbuf[:, sl], lg[0 : VOCAB + 4, :])
            nc.sync.dma_start(out[n_steps - 1, :, :], lgbuf[:, :])

    nc.finalize()
    return nc


def _prep_weights(
    w_proj,
    b_proj,
    wih0,
    whh0,
    bih0,
    bhh0,
    wih1,
    whh1,
    bih1,
    bhh1,
    wih2,
    whh2,
    bih2,
    bhh2,
    w_out,
    b_out,
):
    """Host-side packing: transpose weights, fold b_proj into layer-0 input
    bias, append bias rows, pack w_out into the layer-2 n-gate stationary."""
    bf16 = ml_dtypes.bfloat16
    f32 = np.float32

    def stat(w, b):
        # [out, in] weight + [out] bias -> [in+1, out] stationary
        return np.concatenate([w.T, b[None, :]], axis=0).astype(bf16)

    bih0_eff = (bih0 + wih0 @ b_proj).astype(f32)
    wihT = [stat(wih0, bih0_eff), stat(wih1, bih1), stat(wih2, bih2)]
    whhT = [stat(whh0, bhh0), stat(whh1, bhh1), stat(whh2, bhh2)]
    wout_stat = stat(w_out, b_out)  # [101, 20]
    wnout = np.concatenate([whhT[2][:, 2 * H : 3 * H], wout_stat], axis=1)
    return {
        "wproj": w_proj.T.astype(bf16),
        "wih0": wihT[0],
        "wih1": wihT[1],
        "wih2": wihT[2],
        "whh0": whhT[0],
        "whh1": whhT[1],
        "whh2": whhT[2],
        "wnout": np.ascontiguousarray(wnout).astype(bf16),
    }


_NC_CACHE = {}


def _get_nc():
    if "nc" not in _NC_CACHE:
        _NC_CACHE["nc"] = build_nc()
    return _NC_CACHE["nc"]


def kernel(
    encoder_hidden,
    w_proj,
    b_proj,
    wih0,
    whh0,
    bih0,
    bhh0,
    wih1,
    whh1,
    bih1,
    bhh1,
    wih2,
    whh2,
    bih2,
    bhh2,
    w_out,
    b_out,
    _trace=False,
):
    f32 = np.float32
    encoder_hidden = np.asarray(encoder_hidden, f32)
    args = [
        np.asarray(a, f32)
        for a in (
            w_proj,
            b_proj,
            wih0,
            whh0,
            bih0,
            bhh0,
            wih1,
            whh1,
            bih1,
            bhh1,
            wih2,
            whh2,
            bih2,
            bhh2,
            w_out,
            b_out,
        )
    ]
    weights = _prep_weights(*args)

    ehT = np.ascontiguousarray(encoder_hidden.T).astype(ml_dtypes.bfloat16)
    in_maps = []
    for i in range(N_CORES):
        m = dict(weights)
        m["ehT"] = np.ascontiguousarray(ehT[:, i * BS : (i + 1) * BS])
        in_maps.append(m)

    nc = _get_nc()
    res = run_bass_kernel_spmd(
        nc, in_maps, core_ids=list(range(N_CORES)), trace=_trace
    )
    outs = [res.results[i]["out"].transpose(2, 0, 1)[:, :, 4:] for i in range(N_CORES)]
    full = np.concatenate(outs, axis=0)
    if _trace:
        kernel.last_exec_time_ns = res.exec_time_ns
        kernel.last_results = res
    return full


# revision 8
# speedup vs baseline: 1.4092x; 1.4092x over previous
"""Trainium2 Bass kernel for a 3-layer GRU decoder (DecoderRNN).

Math (per timestep, identical input x0 each step):
    x0 = encoder_hidden @ w_proj.T + b_proj
    3 stacked GRU layers (PyTorch gate order r,z,n), then logits = h2 @ w_out.T + b_out

Device mapping (per core, batch shard BS=4096):
  - Layout: features on SBUF partitions, batch on the free dim.
  - Weights pre-transposed/packed on host; biases ride in an extra
    contraction row against a constant 1.0 row held in each state tile.
  - Gate pre-activations accumulate in PSUM; sigmoid(r,z) is one merged
    ACT op per chunk; the n-gate bank is recycled: ghn -> (t1 read) ->
    gxn + identity@t1 -> tanh, so the gxn+r*ghn add runs on the PE.
  - w_out is packed into the layer-2 whh_n stationary; logits(t-1) ride
    the t1 multiply (rows 100:120 against a constant-ones block) straight
    into SBUF, then one DMA per step.
  - Output is [T, 24, BS] bf16 per core (rows 4:24 are logits); host
    transposes/casts back to [B, T, VOCAB] fp32.
"""

import numpy as np
import ml_dtypes

import concourse.bass as bass
import concourse.mybir as mybir
from concourse import bacc
from concourse.tile import TileContext
from concourse.bass_utils import run_bass_kernel_spmd

N_CORES = 8
B = 32768
BS = B // N_CORES  # 4096
LATENT = 128
H = 100
VOCAB = 20
VP = VOCAB + 4  # logits rows padded to a 32-aligned partition window (96:120)
T = 21
C = 512  # batch chunk = one PSUM bank of fp32

BF16 = mybir.dt.bfloat16
F32 = mybir.dt.float32
AF = mybir.ActivationFunctionType
OP = mybir.AluOpType


def build_nc(bs=BS, n_steps=T):
    nchunk = bs // C
    nhalf = bs // 2
    nc = bacc.Bacc("TRN2", target_bir_lowering=False)

    ehT = nc.declare_dram_parameter("ehT", [LATENT, bs], BF16, isOutput=False)
    wproj = nc.declare_dram_parameter("wproj", [LATENT, H], BF16, isOutput=False)
    wih = [
        nc.declare_dram_parameter(f"wih{l}", [H + 1, 3 * H], BF16, isOutput=False)
        for l in range(3)
    ]
    whh = [
        nc.declare_dram_parameter(f"whh{l}", [H + 1, 3 * H], BF16, isOutput=False)
        for l in range(3)
    ]
    # layer-2 n-gate stationary with w_out packed in cols 100:120
    wnout = nc.declare_dram_parameter("wnout", [H + 1, H + VOCAB], BF16, isOutput=False)
    ident = nc.declare_dram_parameter("ident", [H, H], BF16, isOutput=False)
    out = nc.declare_dram_parameter("out", [n_steps, VP, bs], BF16, isOutput=True)

    with TileContext(nc) as tc:
        with (
            tc.tile_pool(name="const", bufs=1) as cpool,
            tc.tile_pool(name="state", bufs=1) as spool,
            tc.tile_pool(name="rz", bufs=2) as rzpool,
            tc.tile_pool(name="work", bufs=2) as wpool,
            tc.tile_pool(name="psum", bufs=2, space="PSUM") as ppool,
            tc.tile_pool(name="psumx", bufs=4, space="PSUM") as xpool,
        ):
            # ---- load weights ----
            eh_sb = cpool.tile([LATENT, bs], BF16, tag="eh")
            nc.sync.dma_start(eh_sb[:, :], ehT[:, :])
            wproj_sb = cpool.tile([LATENT, H], BF16, tag="wproj")
            nc.sync.dma_start(wproj_sb[:, :], wproj[:, :])
            wih_sb = []
            whh_sb = []
            for l in range(3):
                wi = cpool.tile([H + 1, 3 * H], BF16, tag=f"wih{l}")
                nc.sync.dma_start(wi[:, :], wih[l][:, :])
                wih_sb.append(wi)
                wh = cpool.tile([H + 1, 3 * H], BF16, tag=f"whh{l}")
                nc.sync.dma_start(wh[:, :], whh[l][:, :])
                whh_sb.append(wh)
            wnout_sb = cpool.tile([H + 1, H + VOCAB], BF16, tag="wnout")
            nc.sync.dma_start(wnout_sb[:, :], wnout[:, :])
            ident_sb = cpool.tile([H, H], BF16, tag="ident")
            nc.sync.dma_start(ident_sb[:, :], ident[:, :])

            # ---- state tiles ----
            # ones-row writes start at partition 96 (HW requires 32-aligned
            # partition bases), then zero back rows 96:100
            h_sb = []
            for l in range(3):
                h = spool.tile([H + 1, bs], BF16, tag=f"h{l}")
                nc.gpsimd.memset(h[:, :], 0.0)
                nc.gpsimd.memset(h[96 : H + 1, :], 1.0)
                nc.gpsimd.memset(h[96:H, :], 0.0)
                h_sb.append(h)
            x0 = spool.tile([H + 1, bs], BF16, tag="x0")
            nc.gpsimd.memset(x0[96 : H + 1, :], 1.0)
            nc.gpsimd.memset(x0[96:H, :], 0.0)
            gxn0 = spool.tile([H, bs], BF16, tag="gxn0")
            # layer-2 r-buffer with a constant-ones block on rows 100:120
            # (rows 96:100 are rewritten by sigma_r each step)
            r2x = spool.tile([H + VOCAB, bs], BF16, tag="r2x")
            nc.gpsimd.memset(r2x[96 : H + VOCAB, :], 1.0)

            # ---- prologue: x0 = wproj.T @ ehT ; gxn0 = wih0_n.T @ x0 ----
            for c in range(nchunk):
                sl = slice(c * C, (c + 1) * C)
                ps = ppool.tile([H, 2 * C], F32, tag="grz")
                nc.tensor.matmul(
                    ps[:, 0:C], wproj_sb[:, :], eh_sb[:, sl], start=True, stop=True
                )
                nc.scalar.copy(x0[0:H, sl], ps[:, 0:C])
            for c in range(nchunk):
                sl = slice(c * C, (c + 1) * C)
                ps = ppool.tile([H, 2 * C], F32, tag="grz")
                nc.tensor.matmul(
                    ps[:, 0:C],
                    wih_sb[0][:, 2 * H : 3 * H],
                    x0[:, sl],
                    start=True,
                    stop=True,
                )
                nc.scalar.copy(gxn0[:, sl], ps[:, 0:C])

            # ---- time loop ----
            for t in range(n_steps):
                for l in range(3):
                    h = h_sb[l]
                    hprev = x0 if l == 0 else h_sb[l - 1]
                    wi = wih_sb[l]
                    wh = whh_sb[l]

                    t1 = wpool.tile([H + VOCAB if l == 2 else H, bs], BF16, tag="t1")
                    nbuf = wpool.tile([H, bs], BF16, tag="n")
                    dbuf = wpool.tile([H, bs], BF16, tag="d")
                    ebuf = wpool.tile([H, bs], BF16, tag="e")
                    if l == 2:
                        z2 = rzpool.tile([H, bs], BF16, tag="rz")
                    else:
                        rz = rzpool.tile([H, 2 * bs], BF16, tag="rz")
                    if l == 0:
                        t2 = wpool.tile([H, bs], BF16, tag="t2")

                    xq = []
                    for c in range(nchunk):
                        sl = slice(c * C, (c + 1) * C)
                        grz = ppool.tile([H, 2 * C], F32, tag="grz")
                        # r gate (whh part then wih part, accumulated)
                        nc.tensor.matmul(
                            grz[:, 0:C], wh[:, 0:H], h[:, sl], start=True, stop=False
                        )
                        nc.tensor.matmul(
                            grz[:, 0:C], wi[:, 0:H], hprev[:, sl], start=False, stop=True
                        )
                        # z gate
                        nc.tensor.matmul(
                            grz[:, C : 2 * C],
                            wh[:, H : 2 * H],
                            h[:, sl],
                            start=True,
                            stop=False,
                        )
                        nc.tensor.matmul(
                            grz[:, C : 2 * C],
                            wi[:, H : 2 * H],
                            hprev[:, sl],
                            start=False,
                            stop=True,
                        )
                        # n-gate hidden part; layer 2 also computes logits(t-1)
                        gx = xpool.tile([H + VOCAB, C], F32, tag="gx")
                        xq.append(gx)
                        if l == 2:
                            nc.tensor.matmul(
                                gx[:, :], wnout_sb[:, :], h[:, sl], start=True, stop=True
                            )
                            # sigma split so r lands in the ones-extended buffer
                            nc.scalar.activation(
                                r2x[0:H, sl], grz[:, 0:C], AF.Sigmoid
                            )
                            nc.scalar.activation(
                                z2[:, sl], grz[:, C : 2 * C], AF.Sigmoid
                            )
                            # t1 rows 0:100; logits(t-1) ride rows 100:120
                            nc.vector.tensor_mul(
                                t1[:, sl], r2x[:, sl], gx[0 : H + VOCAB, :]
                            )
                        else:
                            nc.tensor.matmul(
                                gx[0:H, :],
                                wh[:, 2 * H : 3 * H],
                                h[:, sl],
                                start=True,
                                stop=True,
                            )
                            nc.scalar.activation(
                                rz[:, c * 2 * C : (c + 1) * 2 * C], grz[:, :], AF.Sigmoid
                            )
                            nc.vector.tensor_mul(
                                t1[:, sl], rz[:, c * 2 * C : c * 2 * C + C], gx[0:H, :]
                            )

                    if l == 0:
                        # n = tanh(t1 + gxn0), full-batch on DVE + ACT
                        nc.vector.tensor_add(t2[:, :], t1[:, :], gxn0[:, :])
                        nc.scalar.activation(nbuf[:, :], t2[:, :], AF.Tanh)
                    else:
                        # recycle the n-gate bank: gxn matmul then identity@t1
                        # accumulates r*ghn on the PE; tanh reads PSUM
                        for c in range(nchunk):
                            sl = slice(c * C, (c + 1) * C)
                            gx = xq[c]
                            nc.tensor.matmul(
                                gx[0:H, :],
                                wi[:, 2 * H : 3 * H],
                                hprev[:, sl],
                                start=True,
                                stop=False,
                            )
                            nc.tensor.matmul(
                                gx[0:H, :],
                                ident_sb[:, :],
                                t1[0:H, sl],
                                start=False,
                                stop=True,
                            )
                            nc.scalar.activation(nbuf[:, sl], gx[0:H, :], AF.Tanh)

                    # h' = n + z*(h - n), in half-batch pieces for pipelining
                    for hf in range(2):
                        hs = slice(hf * nhalf, (hf + 1) * nhalf)
                        nc.vector.tensor_sub(dbuf[:, hs], h[0:H, hs], nbuf[:, hs])
                        if l == 2:
                            nc.vector.tensor_mul(ebuf[:, hs], z2[:, hs], dbuf[:, hs])
                        else:
                            nh = nhalf // C
                            z3 = rz[:, :].rearrange("p (a b) -> p a b", b=2 * C)[
                                :, hf * nh : (hf + 1) * nh, C : 2 * C
                            ]
                            d3 = dbuf[:, hs].rearrange("p (a b) -> p a b", b=C)
                            e3 = ebuf[:, hs].rearrange("p (a b) -> p a b", b=C)
                            nc.vector.tensor_mul(e3, z3, d3)
                        nc.vector.tensor_add(h[0:H, hs], nbuf[:, hs], ebuf[:, hs])

                    if l == 2 and t > 0:
                        nc.sync.dma_start(out[t - 1, :, :], t1[96 : H + VOCAB, :])

            # ---- epilogue: logits for the last step ----
            lgbuf = wpool.tile([VP, bs], BF16, tag="t2")
            for c in range(nchunk):
                sl = slice(c * C, (c + 1) * C)
                lg = xpool.tile([H + VOCAB, C], F32, tag="gx")
                nc.tensor.matmul(
                    lg[:, :], wnout_sb[:, :], h_sb[2][:, sl], start=True, stop=True
                )
                nc.vector.tensor_copy(lgbuf[:, sl], lg[96 : H + VOCAB, :])
            nc.sync.dma_start(out[n_steps - 1, :, :], lgbuf[:, :])

    nc.finalize()
    return nc


def _prep_weights(
    w_proj,
    b_proj,
    wih0,
    whh0,
    bih0,
    bhh0,
    wih1,
    whh1,
    bih1,
    bhh1,
    wih2,
    whh2,
    bih2,
    bhh2,
    w_out,
    b_out,
):
    """Host-side packing: transpose weights, fold b_proj into layer-0 input
    bias, append bias rows, pack w_out into the layer-2 n-gate stationary."""
    bf16 = ml_dtypes.bfloat16
    f32 = np.float32

    def stat(w, b):
        # [out, in] weight + [out] bias -> [in+1, out] stationary
        return np.concatenate([w.T, b[None, :]], axis=0).astype(bf16)

    bih0_eff = (bih0 + wih0 @ b_proj).astype(f32)
    wihT = [stat(wih0, bih0_eff), stat(wih1, bih1), stat(wih2, bih2)]
    whhT = [stat(whh0, bhh0), stat(whh1, bhh1), stat(whh2, bhh2)]
    wout_stat = stat(w_out, b_out)  # [101, 20]
    wnout = np.concatenate([whhT[2][:, 2 * H : 3 * H], wout_stat], axis=1)
    return {
        "wproj": w_proj.T.astype(bf16),
        "wih0": wihT[0],
        "wih1": wihT[1],
        "wih2": wihT[2],
        "whh0": whhT[0],
        "whh1": whhT[1],
        "whh2": whhT[2],
        "wnout": np.ascontiguousarray(wnout).astype(bf16),
        "ident": np.eye(H, dtype=bf16),
    }


_NC_CACHE = {}


def _get_nc():
    if "nc" not in _NC_CACHE:
        _NC_CACHE["nc"] = build_nc()
    return _NC_CACHE["nc"]


def kernel(
    encoder_hidden,
    w_proj,
    b_proj,
    wih0,
    whh0,
    bih0,
    bhh0,
    wih1,
    whh1,
    bih1,
    bhh1,
    wih2,
    whh2,
    bih2,
    bhh2,
    w_out,
    b_out,
    _trace=False,
):
    f32 = np.float32
    encoder_hidden = np.asarray(encoder_hidden, f32)
    args = [
        np.asarray(a, f32)
        for a in (
            w_proj,
            b_proj,
            wih0,
            whh0,
            bih0,
            bhh0,
            wih1,
            whh1,
            bih1,
            bhh1,
            wih2,
            whh2,
            bih2,
            bhh2,
            w_out,
            b_out,
        )
    ]
    weights = _prep_weights(*args)

    ehT = np.ascontiguousarray(encoder_hidden.T).astype(ml_dtypes.bfloat16)
    in_maps = []
    for i in range(N_CORES):
        m = dict(weights)
        m["ehT"] = np.ascontiguousarray(ehT[:, i * BS : (i + 1) * BS])
        in_maps.append(m)

    nc = _get_nc()
    res = run_bass_kernel_spmd(
        nc, in_maps, core_ids=list(range(N_CORES)), trace=_trace
    )
    outs = [
        np.asarray(res.results[i]["out"], f32).transpose(2, 0, 1)[:, :, 4:]
        for i in range(N_CORES)
    ]
    full = np.concatenate(outs, axis=0)
    if _trace:
        kernel.last_exec_time_ns = res.exec_time_ns
        kernel.last_results = res
    return full


# revision 9
# speedup vs baseline: 1.7138x; 1.2161x over previous
"""Trainium2 Bass kernel for a 3-layer GRU decoder (DecoderRNN).

Math (per timestep, identical input x0 each step):
    x0 = encoder_hidden @ w_proj.T + b_proj
    3 stacked GRU layers (PyTorch gate order r,z,n), then logits = h2 @ w_out.T + b_out

Device mapping (per core, batch shard BS=4096):
  - Layout: features on SBUF partitions, batch on the free dim.
  - Weights pre-transposed/packed on host; biases ride in an extra
    contraction row against a constant 1.0 row held in each state tile.
  - Gate pre-activations accumulate in PSUM; sigmoid(r,z) is one merged
    ACT op per chunk; the n-gate bank is recycled: ghn -> (t1 read) ->
    gxn + identity@t1 -> tanh, so the gxn+r*ghn add runs on the PE.
  - w_out is packed into the layer-2 whh_n stationary; logits(t-1) ride
    the t1 multiply (rows 100:120 against a constant-ones block) straight
    into SBUF, then one DMA per step.
  - Output is [T, 24, BS] bf16 per core (rows 4:24 are logits); host
    transposes/casts back to [B, T, VOCAB] fp32.
"""

import numpy as np
import ml_dtypes

import concourse.bass as bass
import concourse.mybir as mybir
from concourse import bacc
from concourse.tile import TileContext
from concourse.bass_utils import run_bass_kernel_spmd

N_CORES = 8
B = 32768
BS = B // N_CORES  # 4096
LATENT = 128
H = 100
VOCAB = 20
VP = VOCAB + 4  # logits rows padded to a 32-aligned partition window (96:120)
T = 21
C = 512  # batch chunk = one PSUM bank of fp32

BF16 = mybir.dt.bfloat16
F32 = mybir.dt.float32
AF = mybir.ActivationFunctionType
OP = mybir.AluOpType


def build_nc(bs=BS, n_steps=T):
    nchunk = bs // C
    nhalf = bs // 2
    nc = bacc.Bacc("TRN2", target_bir_lowering=False)

    ehT = nc.declare_dram_parameter("ehT", [LATENT, bs], BF16, isOutput=False)
    wproj = nc.declare_dram_parameter("wproj", [LATENT, H], BF16, isOutput=False)
    wih = [
        nc.declare_dram_parameter(f"wih{l}", [H + 1, 3 * H], BF16, isOutput=False)
        for l in range(3)
    ]
    whh = [
        nc.declare_dram_parameter(f"whh{l}", [H + 1, 3 * H], BF16, isOutput=False)
        for l in range(3)
    ]
    # layer-2 n-gate stationary with w_out packed in cols 100:120
    wnout = nc.declare_dram_parameter("wnout", [H + 1, H + VOCAB], BF16, isOutput=False)
    ident = nc.declare_dram_parameter("ident", [H, H], BF16, isOutput=False)
    out = nc.declare_dram_parameter("out", [n_steps, VP, bs], BF16, isOutput=True)

    with TileContext(nc) as tc:
        with (
            tc.tile_pool(name="const", bufs=1) as cpool,
            tc.tile_pool(name="state", bufs=1) as spool,
            tc.tile_pool(name="rz", bufs=2) as rzpool,
            tc.tile_pool(name="work", bufs=2) as wpool,
            tc.tile_pool(name="psum", bufs=2, space="PSUM") as ppool,
            tc.tile_pool(name="psumx", bufs=4, space="PSUM") as xpool,
        ):
            # ---- load weights ----
            eh_sb = cpool.tile([LATENT, bs], BF16, tag="eh")
            nc.sync.dma_start(eh_sb[:, :], ehT[:, :])
            wproj_sb = cpool.tile([LATENT, H], BF16, tag="wproj")
            nc.sync.dma_start(wproj_sb[:, :], wproj[:, :])
            wih_sb = []
            whh_sb = []
            for l in range(3):
                wi = cpool.tile([H + 1, 3 * H], BF16, tag=f"wih{l}")
                nc.sync.dma_start(wi[:, :], wih[l][:, :])
                wih_sb.append(wi)
                wh = cpool.tile([H + 1, 3 * H], BF16, tag=f"whh{l}")
                nc.sync.dma_start(wh[:, :], whh[l][:, :])
                whh_sb.append(wh)
            wnout_sb = cpool.tile([H + 1, H + VOCAB], BF16, tag="wnout")
            nc.sync.dma_start(wnout_sb[:, :], wnout[:, :])
            ident_sb = cpool.tile([H, H], BF16, tag="ident")
            nc.sync.dma_start(ident_sb[:, :], ident[:, :])

            # ---- state tiles ----
            # ones-row writes start at partition 96 (HW requires 32-aligned
            # partition bases), then zero back rows 96:100
            h_sb = []
            for l in range(3):
                h = spool.tile([H + 1, bs], BF16, tag=f"h{l}")
                nc.gpsimd.memset(h[:, :], 0.0)
                nc.gpsimd.memset(h[96 : H + 1, :], 1.0)
                nc.gpsimd.memset(h[96:H, :], 0.0)
                h_sb.append(h)
            x0 = spool.tile([H + 1, bs], BF16, tag="x0")
            nc.gpsimd.memset(x0[96 : H + 1, :], 1.0)
            nc.gpsimd.memset(x0[96:H, :], 0.0)
            gxn0 = spool.tile([H, bs], BF16, tag="gxn0")
            # layer-2 r-buffer with a constant-ones block on rows 100:120
            # (rows 96:100 are rewritten by sigma_r each step)
            r2x = spool.tile([H + VOCAB, bs], BF16, tag="r2x")
            nc.gpsimd.memset(r2x[96 : H + VOCAB, :], 1.0)

            # ---- prologue: x0 = wproj.T @ ehT ; gxn0 = wih0_n.T @ x0 ----
            for c in range(nchunk):
                sl = slice(c * C, (c + 1) * C)
                ps = ppool.tile([H, 2 * C], F32, tag="grz")
                nc.tensor.matmul(
                    ps[:, 0:C], wproj_sb[:, :], eh_sb[:, sl], start=True, stop=True
                )
                nc.scalar.copy(x0[0:H, sl], ps[:, 0:C])
            for c in range(nchunk):
                sl = slice(c * C, (c + 1) * C)
                ps = ppool.tile([H, 2 * C], F32, tag="grz")
                nc.tensor.matmul(
                    ps[:, 0:C],
                    wih_sb[0][:, 2 * H : 3 * H],
                    x0[:, sl],
                    start=True,
                    stop=True,
                )
                nc.scalar.copy(gxn0[:, sl], ps[:, 0:C])

            # ---- time loop ----
            for t in range(n_steps):
                for l in range(3):
                    h = h_sb[l]
                    hprev = x0 if l == 0 else h_sb[l - 1]
                    wi = wih_sb[l]
                    wh = whh_sb[l]

                    t1 = wpool.tile([H + VOCAB if l == 2 else H, bs], BF16, tag="t1")
                    nbuf = wpool.tile([H, bs], BF16, tag="n")
                    dbuf = wpool.tile([H, bs], BF16, tag="d")
                    ebuf = wpool.tile([H, bs], BF16, tag="e")
                    if l == 2:
                        z2 = rzpool.tile([H, bs], BF16, tag="rz")
                    else:
                        rz = rzpool.tile([H, 2 * bs], BF16, tag="rz")
                    if l == 0:
                        t2 = wpool.tile([H, bs], BF16, tag="t2")

                    xq = []
                    for c in range(nchunk):
                        sl = slice(c * C, (c + 1) * C)
                        grz = ppool.tile([H, 2 * C], F32, tag="grz")
                        # r gate (whh part then wih part, accumulated)
                        nc.tensor.matmul(
                            grz[:, 0:C], wh[:, 0:H], h[:, sl], start=True, stop=False
                        )
                        nc.tensor.matmul(
                            grz[:, 0:C], wi[:, 0:H], hprev[:, sl], start=False, stop=True
                        )
                        # z gate
                        nc.tensor.matmul(
                            grz[:, C : 2 * C],
                            wh[:, H : 2 * H],
                            h[:, sl],
                            start=True,
                            stop=False,
                        )
                        nc.tensor.matmul(
                            grz[:, C : 2 * C],
                            wi[:, H : 2 * H],
                            hprev[:, sl],
                            start=False,
                            stop=True,
                        )
                        # n-gate hidden part; layer 2 also computes logits(t-1)
                        gx = xpool.tile([H + VOCAB, C], F32, tag="gx")
                        xq.append(gx)
                        if l == 2:
                            nc.tensor.matmul(
                                gx[:, :], wnout_sb[:, :], h[:, sl], start=True, stop=True
                            )
                            # sigma split so r lands in the ones-extended buffer
                            nc.scalar.activation(
                                r2x[0:H, sl], grz[:, 0:C], AF.Sigmoid
                            )
                            nc.scalar.activation(
                                z2[:, sl], grz[:, C : 2 * C], AF.Sigmoid
                            )
                            # t1 rows 0:100; logits(t-1) ride rows 100:120
                            nc.vector.tensor_mul(
                                t1[:, sl], r2x[:, sl], gx[0 : H + VOCAB, :]
                            )
                        else:
                            nc.tensor.matmul(
                                gx[0:H, :],
                                wh[:, 2 * H : 3 * H],
                                h[:, sl],
                                start=True,
                                stop=True,
                            )
                            nc.scalar.activation(
                                rz[:, c * 2 * C : (c + 1) * 2 * C], grz[:, :], AF.Sigmoid
                            )
                            nc.vector.tensor_mul(
                                t1[:, sl], rz[:, c * 2 * C : c * 2 * C + C], gx[0:H, :]
                            )
                            if l == 0:
                                # n = tanh(t1 + gxn0), chunked so it pipelines
                                nc.vector.tensor_add(
                                    t2[:, sl], t1[:, sl], gxn0[:, sl]
                                )
                                nc.scalar.activation(
                                    nbuf[:, sl], t2[:, sl], AF.Tanh
                                )

                    if l >= 1:
                        # recycle the n-gate bank: gxn matmul then identity@t1
                        # accumulates r*ghn on the PE; tanh reads PSUM
                        for c in range(nchunk):
                            sl = slice(c * C, (c + 1) * C)
                            gx = xq[c]
                            nc.tensor.matmul(
                                gx[0:H, :],
                                wi[:, 2 * H : 3 * H],
                                hprev[:, sl],
                                start=True,
                                stop=False,
                            )
                            nc.tensor.matmul(
                                gx[0:H, :],
                                ident_sb[:, :],
                                t1[0:H, sl],
                                start=False,
                                stop=True,
                            )
                            nc.scalar.activation(nbuf[:, sl], gx[0:H, :], AF.Tanh)

                    # h' = n + z*(h - n), in half-batch pieces for pipelining
                    for hf in range(2):
                        hs = slice(hf * nhalf, (hf + 1) * nhalf)
                        nc.vector.tensor_sub(dbuf[:, hs], h[0:H, hs], nbuf[:, hs])
                        if l == 2:
                            nc.vector.tensor_mul(ebuf[:, hs], z2[:, hs], dbuf[:, hs])
                        else:
                            nh = nhalf // C
                            z3 = rz[:, :].rearrange("p (a b) -> p a b", b=2 * C)[
                                :, hf * nh : (hf + 1) * nh, C : 2 * C
                            ]
                            d3 = dbuf[:, hs].rearrange("p (a b) -> p a b", b=C)
                            e3 = ebuf[:, hs].rearrange("p (a b) -> p a b", b=C)
                            nc.vector.tensor_mul(e3, z3, d3)
                        nc.vector.tensor_add(h[0:H, hs], nbuf[:, hs], ebuf[:, hs])

                    if l == 2 and t > 0:
                        nc.sync.dma_start(out[t - 1, :, :], t1[96 : H + VOCAB, :])

            # ---- epilogue: logits for the last step ----
            lgbuf = wpool.tile([VP, bs], BF16, tag="t2")
            for c in range(nchunk):
                sl = slice(c * C, (c + 1) * C)
                lg = xpool.tile([H + VOCAB, C], F32, tag="gx")
                nc.tensor.matmul(
                    lg[:, :], wnout_sb[:, :], h_sb[2][:, sl], start=True, stop=True
                )
                nc.vector.tensor_copy(lgbuf[:, sl], lg[96 : H + VOCAB, :])
            nc.sync.dma_start(out[n_steps - 1, :, :], lgbuf[:, :])

    nc.finalize()
    return nc


def _prep_weights(
    w_proj,
    b_proj,
    wih0,
    whh0,
    bih0,
    bhh0,
    wih1,
    whh1,
    bih1,
    bhh1,
    wih2,
    whh2,
    bih2,
    bhh2,
    w_out,
    b_out,
):
    """Host-side packing: transpose weights, fold b_proj into layer-0 input
    bias, append bias rows, pack w_out into the layer-2 n-gate stationary."""
    bf16 = ml_dtypes.bfloat16
    f32 = np.float32

    def stat(w, b):
        # [out, in] weight + [out] bias -> [in+1, out] stationary
        return np.concatenate([w.T, b[None, :]], axis=0).astype(bf16)

    bih0_eff = (bih0 + wih0 @ b_proj).astype(f32)
    wihT = [stat(wih0, bih0_eff), stat(wih1, bih1), stat(wih2, bih2)]
    whhT = [stat(whh0, bhh0), stat(whh1, bhh1), stat(whh2, bhh2)]
    wout_stat = stat(w_out, b_out)  # [101, 20]
    wnout = np.concatenate([whhT[2][:, 2 * H : 3 * H], wout_stat], axis=1)
    return {
        "wproj": w_proj.T.astype(bf16),
        "wih0": wihT[0],
        "wih1": wihT[1],
        "wih2": wihT[2],
        "whh0": whhT[0],
        "whh1": whhT[1],
        "whh2": whhT[2],
        "wnout": np.ascontiguousarray(wnout).astype(bf16),
        "ident": np.eye(H, dtype=bf16),
    }


_NC_CACHE = {}


def _get_nc():
    if "nc" not in _NC_CACHE:
        _NC_CACHE["nc"] = build_nc()
    return _NC_CACHE["nc"]


def kernel(
    encoder_hidden,
    w_proj,
    b_proj,
    wih0,
    whh0,
    bih0,
    bhh0,
    wih1,
    whh1,
    bih1,
    bhh1,
    wih2,
    whh2,
    bih2,
    bhh2,
    w_out,
    b_out,
    _trace=False,
):
    f32 = np.float32
    encoder_hidden = np.asarray(encoder_hidden, f32)
    args = [
        np.asarray(a, f32)
        for a in (
            w_proj,
            b_proj,
            wih0,
            whh0,
            bih0,
            bhh0,
            wih1,
            whh1,
            bih1,
            bhh1,
            wih2,
            whh2,
            bih2,
            bhh2,
            w_out,
            b_out,
        )
    ]
    weights = _prep_weights(*args)

    ehT = np.ascontiguousarray(encoder_hidden.T).astype(ml_dtypes.bfloat16)
    in_maps = []
    for i in range(N_CORES):
        m = dict(weights)
        m["ehT"] = np.ascontiguousarray(ehT[:, i * BS : (i + 1) * BS])
        in_maps.append(m)

    nc = _get_nc()
    res = run_bass_kernel_spmd(
        nc, in_maps, core_ids=list(range(N_CORES)), trace=_trace
    )
    outs = [
        np.asarray(res.results[i]["out"], f32).transpose(2, 0, 1)[:, :, 4:]
        for i in range(N_CORES)
    ]
    full = np.concatenate(outs, axis=0)
    if _trace:
        kernel.last_exec_time_ns = res.exec_time_ns
        kernel.last_results = res
    return full
